# revision 1
# baseline (speedup 1.0000x reference)
"""log_matmul_exp(x, A) on 8 TRN2 NeuronCores. HW exec ~84 us, rel err ~6e-5.

out[n, e] = logsumexp_d(x[n, d] + A[d, e]) = log(exp(x) @ exp(A))[n, e]

Inputs are standard-normal (|x|, |A| < ~6), so exp() spans ~[e-6, e6] and the
unshifted formulation is exact to fp32 rounding: no max-subtraction needed.

Sharding: 4 shards of N (rows of x / out) x 2 shards of E (cols of A / out),
~20 MB of HBM traffic per core (the minimum over integer grids). x is
transposed on the host so the contraction dim D sits on SBUF partitions, and
both inputs are staged to the device in bf16 (halves load bytes; costs 6e-5
relative error, measured). Per core:
    exT = exp(xT_shard)  [D=1024, ML=1024]  (ACT, bf16 out)
    ea  = exp(A_shard)   [D=1024, EL=2048]  (ACT, bf16 out)
    s   = exT.T @ ea     (PE, bf16 operands at 1 row/cycle, fp32 PSUM accum)
    out = ln(s)          (ACT, fused into the PSUM->SBUF copyback)

Structure notes (hard-won):
- bacc.Bacc + nc.compile() is required: TRN2 instructions support at most ONE
  sync wait; Bacc's generate_event_semaphores splits multi-wait instructions.
- Split-k (kc 0..3 -> PSUM -> SBUF spill; kc 4..7 -> PSUM -> DVE add) keeps
  the PE fed with 32 output tiles of work per arriving input chunk instead of
  idling on the full k-depth of the 8-bank PSUM working set.
- kc outer / nt inner over 4 PSUM banks: 4 consecutive matmuls share each
  stationary weight tile.
- 20 dummy warm-up matmuls while inputs stream in hold the PE's HAM clock
  gate at 8/8 (2.4 GHz; cold is 2x slower) through the real matmul stream.
- Steady-state matmul spacing measures 216 ns = the N=512 bf16 roofline.
"""

import os
import sys

import numpy as np

for _p in ("/opt/trn_rl_repo", "/root/.axon_site/_ro/trn_rl_repo"):
    if os.path.isdir(_p) and _p not in sys.path:
        sys.path.insert(0, _p)

P = 128
D = 1024
N_FULL = 4096
E_FULL = 4096
GRID_N = 4
GRID_E = 2
N_CORES = GRID_N * GRID_E
ML = N_FULL // GRID_N  # 1024 local output rows
EL = E_FULL // GRID_E  # 2048 local output cols
KC = D // P  # 8 contraction chunks
NT = 512  # matmul moving free dim (one PSUM bank of fp32)

IN_BF16 = True

_cache: dict = {}


def _patch_ldw_opt():
    """Enable walrus's LDWEIGHTS optimization (dedups/hides redundant weight
    loads). concourse hardcodes --enable-ldw-opt=false; our inner loops reuse
    each stationary tile across 4 matmuls, so the reload elision matters."""
    if _cache.get("ldw_patched"):
        return
    from concourse import bass_utils

    orig = bass_utils.run_command

    def patched(argv, **kwargs):
        argv = [
            a.replace("--enable-ldw-opt=false", "--enable-ldw-opt=true")
            if isinstance(a, str)
            else a
            for a in argv
        ]
        return orig(argv, **kwargs)

    bass_utils.run_command = patched
    _cache["ldw_patched"] = True


def _build():
    import concourse.tile as tile
    from concourse import bacc, mybir

    AF = mybir.ActivationFunctionType
    f32 = mybir.dt.float32
    bf16 = mybir.dt.bfloat16

    # Bacc (not raw Bass): its compile() runs generate_event_semaphores,
    # which splits multi-wait instructions to satisfy the 1-wait-per-
    # instruction hardware constraint that walrus codegen enforces.
    nc = bacc.Bacc(
        "TRN2",
        target_bir_lowering=False,
        debug=False,
        num_devices=N_CORES,
        num_swdge_queues=4,
        dynamic_dma_scratch_size=256,
    )
    ind = bf16 if IN_BF16 else f32
    xt = nc.dram_tensor("xt", [D, ML], ind, kind="ExternalInput")
    a = nc.dram_tensor("a", [D, EL], ind, kind="ExternalInput")
    out = nc.dram_tensor("out", [ML, EL], f32, kind="ExternalOutput")

    xt3 = xt[:].rearrange("(kc p) m -> p kc m", p=P)
    a3 = a[:].rearrange("(kc p) e -> p kc e", p=P)

    MT = ML // P  # 8 row tiles
    ET = EL // NT  # 4 col tiles
    KH = KC // 2  # split-k: group 0 = kc 0..3, group 1 = kc 4..7

    with tile.TileContext(nc) as tc:
        with (
            tc.tile_pool(name="persist", bufs=1) as persist,
            tc.tile_pool(name="partial", bufs=1) as partial,
            tc.tile_pool(name="outp", bufs=6) as outp,
            tc.tile_pool(name="psum", bufs=8, space="PSUM") as psum_pool,
            tc.tile_pool(name="stage", bufs=8) as stage,
        ):
            # PE warm-up: dummy bf16 matmuls run while the first inputs
            # stream in, so the HAM clock gate reaches 8/8 (2.4 GHz) before
            # the real matmuls start and stays there (cold is 2x slower).
            wm = persist.tile([P, NT], bf16, tag="warm")
            nc.vector.memset(wm[:], 1.0)
            wps = psum_pool.tile([P, NT], f32, tag="ps", name="warm_ps")
            for _ in range(20):
                nc.tensor.matmul(
                    wps[:], lhsT=wm[:, :P], rhs=wm[:], start=True, stop=True
                )

            # Whole-chunk loads (DMA issue on the SP engine costs ~0.6us per
            # instruction, so fewer/bigger transfers win); piecewise exp on
            # the first chunk only, so the first matmul starts early.
            ex = []
            ea = []
            for kc in range(KC):
                st = stage.tile([P, ML], ind, tag="stx")
                nc.sync.dma_start(st[:], xt3[:, kc])
                t = persist.tile([P, ML], bf16, tag=f"ex{kc}")
                if kc == 0:
                    for q in range(0, ML, NT):
                        nc.scalar.activation(
                            t[:, q : q + NT], st[:, q : q + NT], AF.Exp
                        )
                else:
                    nc.scalar.activation(t[:], st[:], AF.Exp)
                ex.append(t)
                su = stage.tile([P, EL], ind, tag="sta")
                nc.sync.dma_start(su[:], a3[:, kc])
                u = persist.tile([P, EL], bf16, tag=f"ea{kc}")
                if kc == 0:
                    for q in range(0, EL, NT):
                        nc.scalar.activation(
                            u[:, q : q + NT], su[:, q : q + NT], AF.Exp
                        )
                else:
                    nc.scalar.activation(u[:], su[:], AF.Exp)
                ea.append(u)

            # Split-k (kc 0..3 spilled to SBUF, kc 4..7 added back) so the PE
            # has work proportional to every arriving input chunk. Within a
            # row tile, kc is OUTER and nt INNER across 4 PSUM banks so 4
            # consecutive matmuls share the same stationary weight tile.
            parts = {}
            for mt in range(MT):
                pss = [
                    psum_pool.tile([P, NT], f32, tag="ps", name=f"ps0_{mt}_{i}")
                    for i in range(ET)
                ]
                for kc in range(KH):
                    for nt in range(ET):
                        nc.tensor.matmul(
                            pss[nt][:],
                            lhsT=ex[kc][:, mt * P : (mt + 1) * P],
                            rhs=ea[kc][:, nt * NT : (nt + 1) * NT],
                            start=(kc == 0),
                            stop=(kc == KH - 1),
                        )
                pt = partial.tile([P, EL], f32, tag=f"pt{mt}")
                parts[mt] = pt
                for nt in range(ET):
                    nc.vector.tensor_copy(pt[:, nt * NT : (nt + 1) * NT], pss[nt][:])

            for mt in range(MT):
                pt = parts[mt]
                pss = [
                    psum_pool.tile([P, NT], f32, tag="ps", name=f"ps1_{mt}_{i}")
                    for i in range(ET)
                ]
                for kc in range(KH, KC):
                    for nt in range(ET):
                        nc.tensor.matmul(
                            pss[nt][:],
                            lhsT=ex[kc][:, mt * P : (mt + 1) * P],
                            rhs=ea[kc][:, nt * NT : (nt + 1) * NT],
                            start=(kc == KH),
                            stop=(kc == KC - 1),
                        )
                # Pipelined epilogue, one 512-wide piece deep: the final sum
                # lands in a fresh contiguous tile, ln runs in place on it,
                # and the store reads the whole tile.
                for nt in range(ET):
                    ob = outp.tile([P, NT], f32, tag="ob", name=f"ob_{mt}_{nt}")
                    nc.vector.tensor_add(
                        ob[:], pss[nt][:], pt[:, nt * NT : (nt + 1) * NT]
                    )
                    nc.scalar.activation(ob[:], ob[:], AF.Ln)
                    nc.sync.dma_start(
                        out[mt * P : (mt + 1) * P, nt * NT : (nt + 1) * NT], ob[:]
                    )
    nc.compile()
    return nc


def _shard_inputs(x: np.ndarray, A: np.ndarray) -> list[dict]:
    if IN_BF16:
        import ml_dtypes

        dt = ml_dtypes.bfloat16
    else:
        dt = np.float32
    xT = np.ascontiguousarray(np.asarray(x).T.astype(dt))  # (D, N)
    A = np.asarray(A).astype(dt)
    in_maps = []
    for c in range(N_CORES):
        i, j = divmod(c, GRID_E)
        in_maps.append(
            {
                "xt": np.ascontiguousarray(xT[:, i * ML : (i + 1) * ML]),
                "a": np.ascontiguousarray(A[:, j * EL : (j + 1) * EL]),
            }
        )
    return in_maps


def _run(x: np.ndarray, A: np.ndarray, trace: bool = False):
    from concourse import bass_utils

    nc = _cache.get("nc")
    if nc is None:
        nc = _build()
        _cache["nc"] = nc

    in_maps = _shard_inputs(np.asarray(x), np.asarray(A))
    res = bass_utils.run_bass_kernel_spmd(
        nc, in_maps, list(range(N_CORES)), trace=trace
    )
    out = np.empty((N_FULL, E_FULL), dtype=np.float32)
    for c in range(N_CORES):
        i, j = divmod(c, GRID_E)
        out[i * ML : (i + 1) * ML, j * EL : (j + 1) * EL] = res.results[c]["out"]
    return out, res


def kernel(x: np.ndarray, A: np.ndarray) -> np.ndarray:
    out, _ = _run(x, A, trace=False)
    return out



# revision 5
# speedup vs baseline: 1.3595x; 1.3595x over previous
"""log_matmul_exp(x, A) on 8 TRN2 NeuronCores — fp8 DoubleRow edition.

out[n, e] = logsumexp_d(x[n, d] + A[d, e]) = log(exp(x) @ exp(A))

Sharding: 4 shards of N x 2 shards of E. Per core: xt [D=1024, ML=1024] and
a [D=1024, EL=2048] arrive bf16; out [ML, EL] leaves bf16 (host -> fp32).

Compute scheme (validated on host, rel err ~2.2e-3 vs 2e-2 gate):
    ex8 = exp(x - 2.5) as fp8e4   (ACT; TRN fp8e4 max normal is 240, so the
                                   shift keeps exp(<=5.5-2.5)=20 in range)
    ea8 = exp(A - 2.5) as fp8e4   (ACT)
    s   = ex8.T @ ea8             (PE, DoubleRow fp8: 2 k-rows/cycle,
                                   [128, 2, m] paired-k layout, fp32 PSUM)
    out = ln(s) + 5.0             (DVE: one tensor_scalar on the fp32 BITS of
                                   PSUM — ln(s) ~= bits(s)*ln2/2^23 - c —
                                   keeps ln off the ACT critical path)

Streaming: x DMAs are sliced into 4 mt-groups so the PE's first batch (2 row
tiles x 4 col tiles across 8 PSUM banks, k-depth 4 accumulated in place) can
start as soon as the first A chunk lands; A-kc0's exp is nt-sliced for the
same reason. 20 dummy DoubleRow matmuls warm the PE clock gate during load.
"""

import os
import sys

import numpy as np

for _p in ("/opt/trn_rl_repo", "/root/.axon_site/_ro/trn_rl_repo"):
    if os.path.isdir(_p) and _p not in sys.path:
        sys.path.insert(0, _p)

P = 128
D = 1024
N_FULL = 4096
E_FULL = 4096
GRID_N = 4
GRID_E = 2
N_CORES = GRID_N * GRID_E
ML = N_FULL // GRID_N  # 1024 local output rows
EL = E_FULL // GRID_E  # 2048 local output cols
KC = D // (2 * P)  # 4 contraction chunks of 256 (paired for DoubleRow)
NT = 512  # matmul moving free dim (one PSUM bank of fp32)
MT = ML // P  # 8 row tiles
ET = EL // NT  # 4 col tiles
XG = 4  # x streamed in 4 mt-group slices
GW = ML // XG  # 256 columns of xt per group (= 2 row tiles)

SHIFT = 2.5  # exp(v - SHIFT); final out = ln(s) + 2*SHIFT
LN2 = 0.6931471805599453
# ln(s) ~= bits(s) * LN2/2^23 - (127 - eps)*LN2, eps = mean of log2(1+t)-t
LN_EPS = 0.0573
LN_S1 = LN2 / (1 << 23)
LN_S2 = 2.0 * SHIFT - (127.0 - LN_EPS) * LN2

LN_MODE = "dve_bits"  # "dve_bits" | "act"

_cache: dict = {}


def _patch_ldw_opt():
    """Enable walrus's LDWEIGHTS optimization (dedups/hides redundant weight
    loads). concourse hardcodes --enable-ldw-opt=false; our inner loops reuse
    each stationary tile across 4 matmuls, so the reload elision matters."""
    if _cache.get("ldw_patched"):
        return
    from concourse import bass_utils

    orig = bass_utils.run_command

    def patched(argv, **kwargs):
        argv = [
            a.replace("--enable-ldw-opt=false", "--enable-ldw-opt=true")
            if isinstance(a, str)
            else a
            for a in argv
        ]
        return orig(argv, **kwargs)

    bass_utils.run_command = patched
    _cache["ldw_patched"] = True


def _build():
    import concourse.tile as tile
    from concourse import bacc, mybir

    AF = mybir.ActivationFunctionType
    DR = mybir.MatmulPerfMode.DoubleRow
    f32 = mybir.dt.float32
    bf16 = mybir.dt.bfloat16
    f8 = mybir.dt.float8e4
    i32 = mybir.dt.int32

    nc = bacc.Bacc(
        "TRN2",
        target_bir_lowering=False,
        debug=False,
        num_devices=N_CORES,
        num_swdge_queues=4,
        dynamic_dma_scratch_size=256,
    )
    xt = nc.dram_tensor("xt", [D, ML], bf16, kind="ExternalInput")
    a = nc.dram_tensor("a", [D, EL], bf16, kind="ExternalInput")
    out = nc.dram_tensor("out", [ML, EL], bf16, kind="ExternalOutput")

    # d = kc*256 + sub*128 + p: paired-k layout for DoubleRow matmuls.
    xt3 = xt[:].rearrange("(kc sub p) m -> p kc sub m", p=P, sub=2)
    a3 = a[:].rearrange("(kc sub p) e -> p kc sub e", p=P, sub=2)

    with tile.TileContext(nc) as tc:
        with (
            tc.tile_pool(name="persist", bufs=1) as persist,
            tc.tile_pool(name="outp", bufs=4) as outp,
            tc.tile_pool(name="psum", bufs=8, space="PSUM") as psum_pool,
            tc.tile_pool(name="stage", bufs=4) as stage,
        ):
            # PE warm-up while inputs stream in: holds the HAM clock gate at
            # 8/8 (2.4 GHz) through the real matmul stream.
            wm = persist.tile([P, 2, NT], f8, tag="warm")
            nc.vector.memset(wm[:], 1.0)
            nbias = persist.tile([P, 1], f32, tag="nbias")
            nc.vector.memset(nbias[:], -SHIFT)
            wps = psum_pool.tile([P, NT], f32, tag="ps", name="warm_ps")
            for _ in range(20):
                nc.tensor.matmul(
                    wps[:],
                    lhsT=wm[:, :, :P],
                    rhs=wm[:],
                    start=True,
                    stop=True,
                    perf_mode=DR,
                )

            # --- input staging -------------------------------------------
            stx = [
                stage.tile([P, 2, ML], bf16, tag="stx", name=f"stx{k}")
                for k in range(KC)
            ]
            sta = [
                stage.tile([P, 2, EL], bf16, tag="sta", name=f"sta{k}")
                for k in range(KC)
            ]
            ex8 = [
                persist.tile([P, 2, ML], f8, tag=f"ex{k}", name=f"ex8_{k}")
                for k in range(KC)
            ]
            ea8 = [
                persist.tile([P, 2, EL], f8, tag=f"ea{k}", name=f"ea8_{k}")
                for k in range(KC)
            ]

            def dma_xg(g):
                sl = slice(g * GW, (g + 1) * GW)
                for kc in range(KC):
                    nc.sync.dma_start(stx[kc][:, :, sl], xt3[:, kc, :, sl])

            def exp_xg(g):
                sl = slice(g * GW, (g + 1) * GW)
                for kc in range(KC):
                    nc.scalar.activation(
                        ex8[kc][:, :, sl], stx[kc][:, :, sl], AF.Exp, bias=nbias[:]
                    )

            # DMA issue order = ACT consume order: first x-group, then all of
            # A (the whole of A gates the PE's first batch), then x tail.
            dma_xg(0)
            for kc in range(KC):
                nc.sync.dma_start(sta[kc][:], a3[:, kc])
            for g in range(1, XG):
                dma_xg(g)

            exp_xg(0)
            for kc in range(KC):
                if kc == 0:  # nt-sliced so the first matmul starts early
                    for q in range(0, EL, NT):
                        nc.scalar.activation(
                            ea8[kc][:, :, q : q + NT],
                            sta[kc][:, :, q : q + NT],
                            AF.Exp,
                            bias=nbias[:],
                        )
                else:
                    nc.scalar.activation(
                        ea8[kc][:], sta[kc][:], AF.Exp, bias=nbias[:]
                    )
            for g in range(1, XG):
                exp_xg(g)

            # --- matmul batches + epilogue -------------------------------
            # Batch = 2 row tiles x 4 col tiles = 8 PSUM banks; k-depth 4
            # accumulated in place (no split-k spill: GpSimd has no PSUM
            # port, so spill+add would overload DVE/ACT).
            obs = {}
            for b in range(MT // 2):
                mts = (2 * b, 2 * b + 1)
                pss = [
                    psum_pool.tile([P, NT], f32, tag="ps", name=f"ps_{b}_{t}")
                    for t in range(8)
                ]
                for kc in range(KC):
                    for t in range(8):
                        mt = mts[t // 4]
                        ntl = t % 4
                        nc.tensor.matmul(
                            pss[t][:],
                            lhsT=ex8[kc][:, :, mt * P : (mt + 1) * P],
                            rhs=ea8[kc][:, :, ntl * NT : (ntl + 1) * NT],
                            start=(kc == 0),
                            stop=(kc == KC - 1),
                            perf_mode=DR,
                        )
                for mt in mts:
                    obs[mt] = outp.tile([P, EL], bf16, tag="ob", name=f"ob_{mt}")
                for t in range(8):
                    mt = mts[t // 4]
                    ntl = t % 4
                    osl = obs[mt][:, ntl * NT : (ntl + 1) * NT]
                    if LN_MODE == "dve_bits":
                        nc.vector.tensor_scalar(
                            out=osl,
                            in0=pss[t][:].bitcast(i32),
                            scalar1=LN_S1,
                            scalar2=LN_S2,
                            op0=mybir.AluOpType.mult,
                            op1=mybir.AluOpType.add,
                        )
                    else:
                        # ln(s * e^{2c}) = ln(s) + 2c, fused via input scale
                        nc.scalar.activation(
                            osl, pss[t][:], AF.Ln, scale=float(np.exp(2.0 * SHIFT))
                        )
                for mt in mts:
                    nc.sync.dma_start(out[mt * P : (mt + 1) * P, :], obs[mt][:])
    nc.compile()
    return nc


def _shard_inputs(x: np.ndarray, A: np.ndarray) -> list[dict]:
    import ml_dtypes

    dt = ml_dtypes.bfloat16
    xT = np.ascontiguousarray(np.asarray(x).T.astype(dt))  # (D, N)
    A = np.asarray(A).astype(dt)
    in_maps = []
    for c in range(N_CORES):
        i, j = divmod(c, GRID_E)
        in_maps.append(
            {
                "xt": np.ascontiguousarray(xT[:, i * ML : (i + 1) * ML]),
                "a": np.ascontiguousarray(A[:, j * EL : (j + 1) * EL]),
            }
        )
    return in_maps


def _run(x: np.ndarray, A: np.ndarray, trace: bool = False):
    from concourse import bass_utils

    # NOTE: _patch_ldw_opt (--enable-ldw-opt=true) is NOT called: walrus
    # rejects DoubleRow InstLdweights under LDW optimization.
    nc = _cache.get("nc")
    if nc is None:
        nc = _build()
        _cache["nc"] = nc

    in_maps = _shard_inputs(np.asarray(x), np.asarray(A))
    res = bass_utils.run_bass_kernel_spmd(
        nc, in_maps, list(range(N_CORES)), trace=trace
    )
    out = np.empty((N_FULL, E_FULL), dtype=np.float32)
    for c in range(N_CORES):
        i, j = divmod(c, GRID_E)
        out[i * ML : (i + 1) * ML, j * EL : (j + 1) * EL] = np.asarray(
            res.results[c]["out"]
        ).astype(np.float32)
    return out, res


def kernel(x: np.ndarray, A: np.ndarray) -> np.ndarray:
    out, _ = _run(x, A, trace=False)
    return out
